# revision 1
# baseline (speedup 1.0000x reference)
"""Trainium2 Bass kernel for nn_CrossAttention (B=2, T=V=4096, 16 heads, d=64).

Math: the reference einsums contract the k/v group axis g, so
  weight = softmax((x@Wq) @ (adj @ sum_g Wk_g)^T / sqrt(64))
  out    = (weight @ (adj @ sum_g Wv_g)) @ Wo + bo
The group fold (sum over g of Wk/Wv columns) is done host-side on the
weights; all tensor-sized compute runs on device.

Sharding: 8 cores = (batch b, quarter of T). Each core takes t-rows
[tq*1024, (tq+1)*1024) of batch b, needs adj[b] (redundant across the 4
cores of the same b), and writes its own out slice. No collectives.

Device pipeline per core (all fp32):
  B: stream adj[b] in 256-row stripes -> PE-transpose -> adjT -> K^T
     ([64,4096], zero-padded to 128 partitions) and V~ ([v,65] tiles,
     col 64 = ones so P@V also yields softmax denominators).
  C: same for x slice -> q^T per head, zero-padded to K=128 so every
     matmul runs in the PE's (128,128) tile mode (no mode switches).
  D: per (t-half, 4-head group): for each of 32 v-blocks, 4 S^T matmuls
     into one [128,2048] PSUM tile, a single Exp on ACT (scale=1/8
     folded in), then 4 P@V matmuls accumulating O^T[65,512] per head.
     Row 64 of O^T = softmax sum; reciprocal + broadcast-multiply
     normalizes into attnT.
  E: out-proj from attnT with Wo, bias add, DMA out.
"""

import numpy as np

import concourse.bass as bass
import concourse.tile as tile
from concourse import bacc, mybir
from concourse.masks import make_identity

F32 = mybir.dt.float32
F32R = mybir.dt.float32r


def _r(ap):
    return ap.bitcast(F32R)

# Problem constants (hardcoded per the harness contract).
B = 2
T = 4096
V = 4096
E = 1024     # n_embd
HID = 1024   # n_hidden
NH = 16
DH = 64
G = 4
N_CORES = 8
T_CORE = (B * T) // N_CORES  # 1024 t-rows per core
P = 128

# Tiling parameters.
T_TILE = 512          # t-columns per attention tile (fp32 matmul N max)
HPG = 4               # heads per group (4 S banks + 4 O banks = 8 PSUM banks)
ROW_G = 256           # rows per transpose stripe in build phases
SCALE = 1.0 / 8.0     # 1/sqrt(DH)


def build_nc():
    """Build the per-core Bass program (same program on all 8 cores)."""
    EB = E // P                # 8  e-blocks
    DB = HID // P              # 8  dq-blocks
    NVB = V // P               # 32 v-blocks
    NTT = T_CORE // T_TILE     # 2  t-halves
    NHG = NH // HPG            # 4  head groups
    GC = ROW_G // P            # 2  128-row chunks per stripe
    NSTRIPE_V = V // ROW_G     # 16
    NSTRIPE_T = T_TILE // ROW_G  # 2 stripes per t-half

    nc = bacc.Bacc("TRN2", target_bir_lowering=False, debug=False,
                   num_devices=N_CORES)

    x_sl = nc.declare_dram_parameter("x_sl", [T_CORE, E], F32, isOutput=False)
    adj_b = nc.declare_dram_parameter("adj_b", [V, E], F32, isOutput=False)
    Wq = nc.declare_dram_parameter("Wq", [E, HID], F32R, isOutput=False)
    bq = nc.declare_dram_parameter("bq", [HID], F32, isOutput=False)
    Wk = nc.declare_dram_parameter("Wk", [E, DH], F32R, isOutput=False)
    bk = nc.declare_dram_parameter("bk", [DH], F32, isOutput=False)
    Wv = nc.declare_dram_parameter("Wv", [E, DH], F32R, isOutput=False)
    bv = nc.declare_dram_parameter("bv", [DH], F32, isOutput=False)
    Wo = nc.declare_dram_parameter("Wo", [HID, HID], F32R, isOutput=False)
    bo = nc.declare_dram_parameter("bo", [HID], F32, isOutput=False)
    out_sl = nc.declare_dram_parameter("out_sl", [T_CORE, HID], F32,
                                       isOutput=True)
    # DRAM bounce buffer for partition-broadcasting softmax reciprocals.
    sums_dram = nc.dram_tensor("sums_scratch", [NH, T_CORE], F32)

    def bcast_ap(param, n_part, n_free):
        a = param[:] if not isinstance(param, bass.AP) else param
        return bass.AP(tensor=a.tensor, offset=a.offset,
                       ap=[[0, n_part]] + list(a.ap))

    from contextlib import ExitStack
    with tile.TileContext(nc, pool_alloc_mode="queue") as tc, ExitStack() as st:
        consts = st.enter_context(tc.tile_pool(name="consts", bufs=1))
        persist = st.enter_context(tc.tile_pool(name="persist", bufs=1))

        ident = consts.tile([P, P], F32)
        make_identity(nc, ident[:])
        bq_sb = consts.tile([P, DB], F32)
        nc.sync.dma_start(bq_sb[:], bq.rearrange("(db dp) -> dp db", dp=P))
        bk_sb = consts.tile([DH, 1], F32)
        nc.sync.dma_start(bk_sb[:], bk.rearrange("(a one) -> a one", one=1))
        bvb = consts.tile([P, DH], F32)
        nc.gpsimd.dma_start(bvb[:], bcast_ap(bv, P, DH))
        bob = consts.tile([P, HID], F32)
        nc.gpsimd.dma_start(bob[:], bcast_ap(bo, P, HID))

        # Persistent operands of the attention phase.
        kT = persist.tile([P, V], F32R)          # K^T, rows 64..127 zero
        vt = persist.tile([P, NVB, DH + 1], F32R)  # V~ per v-block + ones col
        qT = persist.tile([P, NH, T_CORE], F32R)   # q^T per head, zero-padded
        attnT = persist.tile([P, DB, T_CORE], F32R)  # normalized O^T
        nc.gpsimd.memset(kT[DH:P, :].bitcast(F32), 0.0)
        nc.gpsimd.memset(qT[DH:P, :, :].bitcast(F32), 0.0)
        nc.gpsimd.memset(vt[:, :, DH:DH + 1].bitcast(F32), 1.0)

        # ---- Phase B: K^T and V~ from adj ----
        with (
            tc.tile_pool(name="bwork", bufs=2) as bw,
            tc.tile_pool(name="bw1", bufs=1) as bw1,
            tc.tile_pool(name="bpsum", bufs=2, space="PSUM") as bp,
        ):
            # Wk padded to 128 cols so the K-proj output is [128, N].
            Wk_sb = bw1.tile([P, EB, P], F32R)
            nc.gpsimd.memset(Wk_sb[:, :, DH:P].bitcast(F32), 0.0)
            nc.sync.dma_start(Wk_sb[:, :, 0:DH],
                             Wk.rearrange("(eb ep) d -> ep eb d", ep=P))
            Wv_sb = bw1.tile([P, EB, DH], F32R)
            nc.sync.dma_start(Wv_sb[:],
                             Wv.rearrange("(eb ep) d -> ep eb d", ep=P))

            for sv in range(NSTRIPE_V):
                r0 = sv * ROW_G
                adj_in = bw.tile([P, GC, E], F32, tag="row_in")
                nc.sync.dma_start(
                    adj_in[:],
                    adj_b[r0:r0 + ROW_G, :].rearrange("(c p) e -> p c e", p=P))
                aT = bw.tile([P, EB, ROW_G], F32R, tag="aT")
                for eb in range(EB):
                    for cc in range(GC):
                        ptr = bp.tile([P, P], F32, tag="ptr")
                        nc.tensor.transpose(
                            ptr[:], adj_in[:, cc, eb * P:(eb + 1) * P], ident[:])
                        nc.vector.tensor_copy(aT[:, eb, cc * P:(cc + 1) * P],
                                              ptr[:])
                # K^T columns for this stripe.
                pk = bp.tile([P, ROW_G], F32, tag="pk")
                for eb in range(EB):
                    nc.tensor.matmul(pk[:], Wk_sb[:, eb, :], aT[:, eb, :],
                                     start=(eb == 0), stop=(eb == EB - 1))
                nc.vector.tensor_scalar_add(kT[0:DH, r0:r0 + ROW_G],
                                            pk[0:DH, :], bk_sb[:])
                # V~ rows for this stripe.
                for cc in range(GC):
                    vb = (r0 + cc * P) // P
                    pv = bp.tile([P, DH], F32, tag="pv")
                    for eb in range(EB):
                        nc.tensor.matmul(pv[:], aT[:, eb, cc * P:(cc + 1) * P],
                                         Wv_sb[:, eb, :],
                                         start=(eb == 0), stop=(eb == EB - 1))
                    nc.vector.tensor_add(vt[:, vb, 0:DH], pv[:], bvb[:])

        # ---- Phase C: q^T from x ----
        with (
            tc.tile_pool(name="cwork", bufs=2) as cw,
            tc.tile_pool(name="cw1", bufs=1) as cw1,
            tc.tile_pool(name="cpsum", bufs=2, space="PSUM") as cp,
        ):
            for tt in range(NTT):
                xT = cw1.tile([P, EB, T_TILE], F32R, tag="xT")
                for st in range(NSTRIPE_T):
                    r0 = tt * T_TILE + st * ROW_G
                    x_in = cw.tile([P, GC, E], F32, tag="x_in")
                    nc.sync.dma_start(
                        x_in[:],
                        x_sl[r0:r0 + ROW_G, :]
                        .rearrange("(c p) e -> p c e", p=P))
                    for eb in range(EB):
                        for cc in range(GC):
                            ptr = cp.tile([P, P], F32, tag="ptr")
                            nc.tensor.transpose(
                                ptr[:], x_in[:, cc, eb * P:(eb + 1) * P],
                                ident[:])
                            nc.vector.tensor_copy(
                                xT[:, eb, st * ROW_G + cc * P:
                                   st * ROW_G + (cc + 1) * P], ptr[:])
                ts0 = tt * T_TILE
                for half in range(2):
                    db_lo = half * (DB // 2)
                    Wq_sb = cw1.tile([P, EB, (DB // 2) * P], F32R, tag="wq")
                    nc.sync.dma_start(
                        Wq_sb[:],
                        Wq[:, db_lo * P:(db_lo + DB // 2) * P]
                        .rearrange("(eb ep) d -> ep eb d", ep=P))
                    for dbr in range(DB // 2):
                        db = db_lo + dbr
                        pq = cp.tile([P, T_TILE], F32, tag="pq")
                        for eb in range(EB):
                            nc.tensor.matmul(
                                pq[:], Wq_sb[:, eb, dbr * P:(dbr + 1) * P],
                                xT[:, eb, :],
                                start=(eb == 0), stop=(eb == EB - 1))
                        # head 2*db from partitions 0..63 (same-lane copy)
                        nc.vector.tensor_scalar_add(
                            qT[0:DH, 2 * db, ts0:ts0 + T_TILE],
                            pq[0:DH, :], bq_sb[0:DH, db:db + 1])
                        # head 2*db+1 from partitions 64..127 (via DMA)
                        qtmp = cw.tile([P, T_TILE], F32R, tag="qtmp")
                        nc.vector.tensor_scalar_add(
                            qtmp[DH:P, :], pq[DH:P, :], bq_sb[DH:P, db:db + 1])
                        nc.gpsimd.dma_start(
                            qT[0:DH, 2 * db + 1, ts0:ts0 + T_TILE],
                            qtmp[DH:P, :])

        # ---- Phase D: attention ----
        # PSUM: 2 x S2[128,1024] (2 banks each, double-buffered) + 4 x O
        # banks = 8.  Each exp covers a 2-head [128,1024] tile so ACT
        # streams continuously while PE fills the other S2 buffer.
        with (
            tc.tile_pool(name="dwork", bufs=3) as dw,
            tc.tile_pool(name="dnorm", bufs=1) as dn,
            tc.tile_pool(name="dpsum", bufs=2, space="PSUM") as dps,
            tc.tile_pool(name="opsum", bufs=1, space="PSUM") as ops,
        ):
            for tt in range(NTT):
                ts0 = tt * T_TILE
                for hg in range(NHG):
                    heads = [hg * HPG + i for i in range(HPG)]
                    h0 = heads[0]
                    O4t = ops.tile([DH + 1, HPG, T_TILE], F32, tag="O4t")
                    for vb in range(NVB):
                        P2s = []
                        for pp in range(HPG // 2):
                            S2 = dps.tile([P, 2 * T_TILE], F32, tag="S2")
                            for h2 in range(2):
                                hi = pp * 2 + h2
                                nc.tensor.matmul(
                                    S2[:, h2 * T_TILE:(h2 + 1) * T_TILE],
                                    kT[:, vb * P:(vb + 1) * P],
                                    qT[:, heads[hi], ts0:ts0 + T_TILE],
                                    start=True, stop=True)
                            P2 = dw.tile([P, 2 * T_TILE], F32R, tag="P2")
                            nc.scalar.activation(
                                P2[:], S2[:],
                                mybir.ActivationFunctionType.Exp, scale=SCALE)
                            P2s.append(P2)
                            if vb > 0:
                                for h2 in range(2):
                                    hi = pp * 2 + h2
                                    nc.tensor.matmul(
                                        O4t[:, hi, :], vt[:, vb, :],
                                        P2[:, h2 * T_TILE:(h2 + 1) * T_TILE],
                                        start=False, stop=(vb == NVB - 1),
                                        skip_group_check=True)
                        if vb == 0:
                            # First v-block: all S+exp before any P@V so the
                            # PE isn't stalled behind the O-bank release.
                            for pp in range(HPG // 2):
                                for h2 in range(2):
                                    hi = pp * 2 + h2
                                    nc.tensor.matmul(
                                        O4t[:, hi, :], vt[:, vb, :],
                                        P2s[pp][:, h2 * T_TILE:
                                                (h2 + 1) * T_TILE],
                                        start=True, stop=False,
                                        skip_group_check=True)
                    # Normalize: row DH of O4t holds the softmax denominator.
                    # One evacuation frees all 4 O banks; reciprocal runs off
                    # the critical path on broadcast data.
                    onorm = dn.tile([DH + 1, HPG, T_TILE], F32, tag="onorm")
                    nc.vector.tensor_copy(onorm[:], O4t[:])
                    nc.gpsimd.dma_start(
                        sums_dram[h0:h0 + HPG, ts0:ts0 + T_TILE],
                        onorm[DH:DH + 1, :, :])
                    sbc = dn.tile([DH, HPG, T_TILE], F32, tag="sbc")
                    nc.gpsimd.dma_start(
                        sbc[:],
                        bcast_ap(sums_dram[h0:h0 + HPG, ts0:ts0 + T_TILE],
                                 DH, HPG * T_TILE))
                    rec = dn.tile([DH, HPG, T_TILE], F32, tag="rec")
                    nc.vector.reciprocal_approx_fast(rec[:], sbc[:])
                    for hi, h in enumerate(heads):
                        db = h // 2
                        if h % 2 == 0:
                            nc.vector.tensor_mul(
                                attnT[0:DH, db, ts0:ts0 + T_TILE],
                                onorm[0:DH, hi, :], rec[:, hi, :])
                        else:
                            nrm = dn.tile([DH, T_TILE], F32, tag="nrm")
                            nc.vector.tensor_mul(nrm[:], onorm[0:DH, hi, :],
                                                 rec[:, hi, :])
                            nc.gpsimd.dma_start(
                                attnT[DH:P, db, ts0:ts0 + T_TILE], nrm[:])

        # ---- Phase E: output projection ----
        with (
            tc.tile_pool(name="ework", bufs=3) as ew,
            tc.tile_pool(name="ew1", bufs=1) as ew1,
            tc.tile_pool(name="epsum", bufs=2, space="PSUM") as ep,
        ):
            Wo_sb = ew1.tile([P, DB, HID], F32R)
            nc.sync.dma_start(Wo_sb[:],
                             Wo.rearrange("(kb kp) e -> kp kb e", kp=P))
            for tc_i in range(T_CORE // P):
                for eh in range(HID // T_TILE):
                    po = ep.tile([P, T_TILE], F32, tag="po")
                    for kb in range(DB):
                        nc.tensor.matmul(
                            po[:], attnT[:, kb, tc_i * P:(tc_i + 1) * P],
                            Wo_sb[:, kb, eh * T_TILE:(eh + 1) * T_TILE],
                            start=(kb == 0), stop=(kb == DB - 1))
                    ot = ew.tile([P, T_TILE], F32, tag="ot")
                    nc.vector.tensor_add(
                        ot[:], po[:], bob[:, eh * T_TILE:(eh + 1) * T_TILE])
                    nc.sync.dma_start(
                        out_sl[tc_i * P:(tc_i + 1) * P,
                               eh * T_TILE:(eh + 1) * T_TILE], ot[:])

    nc.compile()
    return nc


_NC = None


def _get_nc():
    global _NC
    if _NC is None:
        _NC = build_nc()
    return _NC


def kernel(x, adj, Wq, bq, Wk, bk, Wv, bv, Wo, bo):
    x = np.asarray(x, np.float32)
    adj = np.asarray(adj, np.float32)
    Wq_f = np.ascontiguousarray(np.asarray(Wq, np.float32))
    bq_f = np.ascontiguousarray(np.asarray(bq, np.float32))
    Wk_f = np.ascontiguousarray(
        np.asarray(Wk, np.float32).reshape(E, G, DH).sum(axis=1))
    bk_f = np.ascontiguousarray(
        np.asarray(bk, np.float32).reshape(G, DH).sum(axis=0))
    Wv_f = np.ascontiguousarray(
        np.asarray(Wv, np.float32).reshape(E, G, DH).sum(axis=1))
    bv_f = np.ascontiguousarray(
        np.asarray(bv, np.float32).reshape(G, DH).sum(axis=0))
    Wo_f = np.ascontiguousarray(np.asarray(Wo, np.float32))
    bo_f = np.ascontiguousarray(np.asarray(bo, np.float32))

    nc = _get_nc()
    in_maps = []
    for c in range(N_CORES):
        b = c // (N_CORES // B)
        tq = c % (N_CORES // B)
        in_maps.append({
            "x_sl": np.ascontiguousarray(
                x[b, tq * T_CORE:(tq + 1) * T_CORE, :]),
            "adj_b": np.ascontiguousarray(adj[b]),
            "Wq": Wq_f, "bq": bq_f, "Wk": Wk_f, "bk": bk_f,
            "Wv": Wv_f, "bv": bv_f, "Wo": Wo_f, "bo": bo_f,
        })

    from concourse.bass_utils import run_bass_kernel_spmd
    res = run_bass_kernel_spmd(nc, in_maps, list(range(N_CORES)))

    out = np.empty((B, T, HID), np.float32)
    for c in range(N_CORES):
        b = c // (N_CORES // B)
        tq = c % (N_CORES // B)
        out[b, tq * T_CORE:(tq + 1) * T_CORE, :] = res.results[c]["out_sl"]
    return out



# revision 2
# speedup vs baseline: 1.1082x; 1.1082x over previous
"""Trainium2 Bass kernel for nn_CrossAttention (B=2, T=V=4096, 16 heads, d=64).

Math: the reference einsums contract the k/v group axis g, so
  weight = softmax((x@Wq) @ (adj @ sum_g Wk_g)^T / sqrt(64))
  out    = (weight @ (adj @ sum_g Wv_g)) @ Wo + bo
The group fold (sum over g of Wk/Wv columns) is done host-side on the
weights; all tensor-sized compute runs on device.

Sharding: 8 cores = (batch b, quarter of T). Each core takes t-rows
[tq*1024, (tq+1)*1024) of batch b, needs adj[b] (redundant across the 4
cores of the same b), and writes its own out slice. No collectives.

All matmul operands are bf16 (cast host-side for DRAM inputs); PSUM
accumulation stays fp32 and the softmax normalize + output are fp32.
bf16 halves SBUF/DMA traffic and enables the PE's fast weight load.

Device pipeline per core:
  B: stream adj[b] in 256-row stripes -> PE-transpose -> adjT -> K^T
     ([64,4096], zero-padded to 128 partitions) and V~ ([v,65] tiles,
     col 64 = ones so P@V also yields softmax denominators).
  C: same for x slice -> q^T per head, zero-padded to K=128.
  D: per (t-half, 4-head group): for each of 32 v-blocks, 4 S^T matmuls
     into one [128,2048] PSUM tile, a single Exp on ACT (scale=1/8
     folded in, bf16 out), then 4 P@V matmuls accumulating O^T[65,512]
     per head.  Row 64 of O^T = softmax sum; reciprocal + broadcast-
     multiply normalizes into attnT.
  E: out-proj from attnT with Wo, bias add, DMA out.
"""

import numpy as np
import ml_dtypes

import concourse.bass as bass
import concourse.tile as tile
from concourse import bacc, mybir
from concourse.masks import make_identity

F32 = mybir.dt.float32
BF16 = mybir.dt.bfloat16
NP_BF16 = ml_dtypes.bfloat16

# Problem constants (hardcoded per the harness contract).
B = 2
T = 4096
V = 4096
E = 1024     # n_embd
HID = 1024   # n_hidden
NH = 16
DH = 64
G = 4
N_CORES = 8
T_CORE = (B * T) // N_CORES  # 1024 t-rows per core
P = 128

# Tiling parameters.
T_TILE = 512          # t-columns per attention tile
HPG = 4               # heads per group (4 S banks + 4 O banks = 8 PSUM banks)
ROW_G = 256           # rows per transpose stripe in build phases
SCALE = 1.0 / 8.0     # 1/sqrt(DH)


def build_nc():
    """Build the per-core Bass program (same program on all 8 cores)."""
    EB = E // P                # 8  e-blocks
    DB = HID // P              # 8  dq-blocks
    NVB = V // P               # 32 v-blocks
    NTT = T_CORE // T_TILE     # 2  t-halves
    NHG = NH // HPG            # 4  head groups
    GC = ROW_G // P            # 2  128-row chunks per stripe
    NSTRIPE_V = V // ROW_G     # 16
    NSTRIPE_T = T_TILE // ROW_G  # 2 stripes per t-half

    nc = bacc.Bacc("TRN2", target_bir_lowering=False, debug=False,
                   num_devices=N_CORES)

    x_sl = nc.declare_dram_parameter("x_sl", [T_CORE, E], BF16, isOutput=False)
    adj_b = nc.declare_dram_parameter("adj_b", [V, E], BF16, isOutput=False)
    Wq = nc.declare_dram_parameter("Wq", [E, HID], BF16, isOutput=False)
    bq = nc.declare_dram_parameter("bq", [HID], F32, isOutput=False)
    Wk = nc.declare_dram_parameter("Wk", [E, DH], BF16, isOutput=False)
    bk = nc.declare_dram_parameter("bk", [DH], F32, isOutput=False)
    Wv = nc.declare_dram_parameter("Wv", [E, DH], BF16, isOutput=False)
    bv = nc.declare_dram_parameter("bv", [DH], F32, isOutput=False)
    Wo = nc.declare_dram_parameter("Wo", [HID, HID], BF16, isOutput=False)
    bo = nc.declare_dram_parameter("bo", [HID], F32, isOutput=False)
    out_sl = nc.declare_dram_parameter("out_sl", [T_CORE, HID], F32,
                                       isOutput=True)
    # DRAM bounce buffer for partition-broadcasting softmax reciprocals.
    sums_dram = nc.dram_tensor("sums_scratch", [NH, T_CORE], F32)

    def bcast_ap(param, n_part, n_free):
        a = param[:] if not isinstance(param, bass.AP) else param
        return bass.AP(tensor=a.tensor, offset=a.offset,
                       ap=[[0, n_part]] + list(a.ap))

    from contextlib import ExitStack
    with tile.TileContext(nc, pool_alloc_mode="queue") as tc, ExitStack() as st:
        consts = st.enter_context(tc.tile_pool(name="consts", bufs=1))
        persist = st.enter_context(tc.tile_pool(name="persist", bufs=1))

        ident = consts.tile([P, P], BF16)
        make_identity(nc, ident[:])
        bq_sb = consts.tile([P, DB], F32)
        nc.sync.dma_start(bq_sb[:], bq.rearrange("(db dp) -> dp db", dp=P))
        bk_sb = consts.tile([DH, 1], F32)
        nc.sync.dma_start(bk_sb[:], bk.rearrange("(a one) -> a one", one=1))
        bvb = consts.tile([P, DH], F32)
        nc.gpsimd.dma_start(bvb[:], bcast_ap(bv, P, DH))
        bob = consts.tile([P, HID], F32)
        nc.gpsimd.dma_start(bob[:], bcast_ap(bo, P, HID))

        # Persistent operands of the attention phase.
        kT = persist.tile([P, V], BF16)          # K^T, rows 64..127 zero
        vt = persist.tile([P, NVB, DH + 1], BF16)  # V~ per v-block + ones col
        qT = persist.tile([P, NH, T_CORE], BF16)   # q^T per head, zero-padded
        attnT = persist.tile([P, DB, T_CORE], BF16)  # normalized O^T
        nc.gpsimd.memset(kT[DH:P, :], 0.0)
        nc.gpsimd.memset(qT[DH:P, :, :], 0.0)
        nc.gpsimd.memset(vt[:, :, DH:DH + 1], 1.0)

        # ---- Phase B: K^T and V~ from adj ----
        with (
            tc.tile_pool(name="bwork", bufs=2) as bw,
            tc.tile_pool(name="bw1", bufs=1) as bw1,
            tc.tile_pool(name="bpsum", bufs=2, space="PSUM") as bp,
        ):
            # Wk padded to 128 cols so the K-proj output is [128, N].
            Wk_sb = bw1.tile([P, EB, P], BF16)
            nc.gpsimd.memset(Wk_sb[:, :, DH:P], 0.0)
            nc.sync.dma_start(Wk_sb[:, :, 0:DH],
                             Wk.rearrange("(eb ep) d -> ep eb d", ep=P))
            Wv_sb = bw1.tile([P, EB, DH], BF16)
            nc.sync.dma_start(Wv_sb[:],
                             Wv.rearrange("(eb ep) d -> ep eb d", ep=P))

            for sv in range(NSTRIPE_V):
                r0 = sv * ROW_G
                adj_in = bw.tile([P, GC, E], BF16, tag="row_in")
                nc.sync.dma_start(
                    adj_in[:],
                    adj_b[r0:r0 + ROW_G, :].rearrange("(c p) e -> p c e", p=P))
                aT = bw.tile([P, EB, ROW_G], BF16, tag="aT")
                for eb in range(EB):
                    for cc in range(GC):
                        ptr = bp.tile([P, P], BF16, tag="ptr")
                        nc.tensor.transpose(
                            ptr[:], adj_in[:, cc, eb * P:(eb + 1) * P], ident[:])
                        nc.vector.tensor_copy(aT[:, eb, cc * P:(cc + 1) * P],
                                              ptr[:])
                # K^T columns for this stripe.
                pk = bp.tile([P, ROW_G], F32, tag="pk")
                for eb in range(EB):
                    nc.tensor.matmul(pk[:], Wk_sb[:, eb, :], aT[:, eb, :],
                                     start=(eb == 0), stop=(eb == EB - 1))
                nc.vector.tensor_scalar_add(kT[0:DH, r0:r0 + ROW_G],
                                            pk[0:DH, :], bk_sb[:])
                # V~ rows for this stripe.
                for cc in range(GC):
                    vb = (r0 + cc * P) // P
                    pv = bp.tile([P, DH], F32, tag="pv")
                    for eb in range(EB):
                        nc.tensor.matmul(pv[:], aT[:, eb, cc * P:(cc + 1) * P],
                                         Wv_sb[:, eb, :],
                                         start=(eb == 0), stop=(eb == EB - 1))
                    nc.vector.tensor_add(vt[:, vb, 0:DH], pv[:], bvb[:])

        # ---- Phase C: q^T from x ----
        with (
            tc.tile_pool(name="cwork", bufs=2) as cw,
            tc.tile_pool(name="cw1", bufs=1) as cw1,
            tc.tile_pool(name="cpsum", bufs=2, space="PSUM") as cp,
        ):
            for tt in range(NTT):
                xT = cw1.tile([P, EB, T_TILE], BF16, tag="xT")
                for st in range(NSTRIPE_T):
                    r0 = tt * T_TILE + st * ROW_G
                    x_in = cw.tile([P, GC, E], BF16, tag="x_in")
                    nc.sync.dma_start(
                        x_in[:],
                        x_sl[r0:r0 + ROW_G, :]
                        .rearrange("(c p) e -> p c e", p=P))
                    for eb in range(EB):
                        for cc in range(GC):
                            ptr = cp.tile([P, P], BF16, tag="ptr")
                            nc.tensor.transpose(
                                ptr[:], x_in[:, cc, eb * P:(eb + 1) * P],
                                ident[:])
                            nc.vector.tensor_copy(
                                xT[:, eb, st * ROW_G + cc * P:
                                   st * ROW_G + (cc + 1) * P], ptr[:])
                ts0 = tt * T_TILE
                for half in range(2):
                    db_lo = half * (DB // 2)
                    Wq_sb = cw1.tile([P, EB, (DB // 2) * P], BF16, tag="wq")
                    nc.sync.dma_start(
                        Wq_sb[:],
                        Wq[:, db_lo * P:(db_lo + DB // 2) * P]
                        .rearrange("(eb ep) d -> ep eb d", ep=P))
                    for dbr in range(DB // 2):
                        db = db_lo + dbr
                        pq = cp.tile([P, T_TILE], F32, tag="pq")
                        for eb in range(EB):
                            nc.tensor.matmul(
                                pq[:], Wq_sb[:, eb, dbr * P:(dbr + 1) * P],
                                xT[:, eb, :],
                                start=(eb == 0), stop=(eb == EB - 1))
                        # head 2*db from partitions 0..63 (same-lane copy)
                        nc.vector.tensor_scalar_add(
                            qT[0:DH, 2 * db, ts0:ts0 + T_TILE],
                            pq[0:DH, :], bq_sb[0:DH, db:db + 1])
                        # head 2*db+1 from partitions 64..127 (via DMA)
                        qtmp = cw.tile([P, T_TILE], BF16, tag="qtmp")
                        nc.vector.tensor_scalar_add(
                            qtmp[DH:P, :], pq[DH:P, :], bq_sb[DH:P, db:db + 1])
                        nc.gpsimd.dma_start(
                            qT[0:DH, 2 * db + 1, ts0:ts0 + T_TILE],
                            qtmp[DH:P, :])

        # ---- Phase D: attention ----
        # PSUM: 2 x S2[128,1024] (2 banks each, double-buffered) + 4 x O
        # banks = 8.  Each exp covers a 2-head [128,1024] tile so ACT
        # streams continuously while PE fills the other S2 buffer.
        with (
            tc.tile_pool(name="dwork", bufs=3) as dw,
            tc.tile_pool(name="dnorm", bufs=1) as dn,
            tc.tile_pool(name="dpsum", bufs=2, space="PSUM") as dps,
            tc.tile_pool(name="opsum", bufs=1, space="PSUM") as ops,
        ):
            for tt in range(NTT):
                ts0 = tt * T_TILE
                for hg in range(NHG):
                    heads = [hg * HPG + i for i in range(HPG)]
                    h0 = heads[0]
                    O4t = ops.tile([DH + 1, HPG, T_TILE], F32, tag="O4t")
                    for vb in range(NVB):
                        P2s = []
                        for pp in range(HPG // 2):
                            S2 = dps.tile([P, 2 * T_TILE], F32, tag="S2")
                            for h2 in range(2):
                                hi = pp * 2 + h2
                                nc.tensor.matmul(
                                    S2[:, h2 * T_TILE:(h2 + 1) * T_TILE],
                                    kT[:, vb * P:(vb + 1) * P],
                                    qT[:, heads[hi], ts0:ts0 + T_TILE],
                                    start=True, stop=True)
                            P2 = dw.tile([P, 2 * T_TILE], BF16, tag="P2")
                            nc.scalar.activation(
                                P2[:], S2[:],
                                mybir.ActivationFunctionType.Exp, scale=SCALE)
                            P2s.append(P2)
                            if vb > 0:
                                for h2 in range(2):
                                    hi = pp * 2 + h2
                                    nc.tensor.matmul(
                                        O4t[:, hi, :], vt[:, vb, :],
                                        P2[:, h2 * T_TILE:(h2 + 1) * T_TILE],
                                        start=False, stop=(vb == NVB - 1),
                                        skip_group_check=True)
                        if vb == 0:
                            # First v-block: all S+exp before any P@V so the
                            # PE isn't stalled behind the O-bank release.
                            for pp in range(HPG // 2):
                                for h2 in range(2):
                                    hi = pp * 2 + h2
                                    nc.tensor.matmul(
                                        O4t[:, hi, :], vt[:, vb, :],
                                        P2s[pp][:, h2 * T_TILE:
                                                (h2 + 1) * T_TILE],
                                        start=True, stop=False,
                                        skip_group_check=True)
                    # Normalize: row DH of O4t holds the softmax denominator.
                    # One evacuation frees all 4 O banks; reciprocal runs off
                    # the critical path on broadcast data.
                    onorm = dn.tile([DH + 1, HPG, T_TILE], F32, tag="onorm")
                    nc.vector.tensor_copy(onorm[:], O4t[:])
                    nc.gpsimd.dma_start(
                        sums_dram[h0:h0 + HPG, ts0:ts0 + T_TILE],
                        onorm[DH:DH + 1, :, :])
                    sbc = dn.tile([DH, HPG, T_TILE], F32, tag="sbc")
                    nc.gpsimd.dma_start(
                        sbc[:],
                        bcast_ap(sums_dram[h0:h0 + HPG, ts0:ts0 + T_TILE],
                                 DH, HPG * T_TILE))
                    rec = dn.tile([DH, HPG, T_TILE], F32, tag="rec")
                    nc.vector.reciprocal_approx_fast(rec[:], sbc[:])
                    for hi, h in enumerate(heads):
                        db = h // 2
                        if h % 2 == 0:
                            nc.vector.tensor_mul(
                                attnT[0:DH, db, ts0:ts0 + T_TILE],
                                onorm[0:DH, hi, :], rec[:, hi, :])
                        else:
                            nrm = dn.tile([DH, T_TILE], BF16, tag="nrm")
                            nc.vector.tensor_mul(nrm[:], onorm[0:DH, hi, :],
                                                 rec[:, hi, :])
                            nc.gpsimd.dma_start(
                                attnT[DH:P, db, ts0:ts0 + T_TILE], nrm[:])

        # ---- Phase E: output projection ----
        with (
            tc.tile_pool(name="ework", bufs=3) as ew,
            tc.tile_pool(name="ew1", bufs=1) as ew1,
            tc.tile_pool(name="epsum", bufs=2, space="PSUM") as ep,
        ):
            Wo_sb = ew1.tile([P, DB, HID], BF16)
            nc.sync.dma_start(Wo_sb[:],
                             Wo.rearrange("(kb kp) e -> kp kb e", kp=P))
            for tc_i in range(T_CORE // P):
                for eh in range(HID // T_TILE):
                    po = ep.tile([P, T_TILE], F32, tag="po")
                    for kb in range(DB):
                        nc.tensor.matmul(
                            po[:], attnT[:, kb, tc_i * P:(tc_i + 1) * P],
                            Wo_sb[:, kb, eh * T_TILE:(eh + 1) * T_TILE],
                            start=(kb == 0), stop=(kb == DB - 1))
                    ot = ew.tile([P, T_TILE], F32, tag="ot")
                    nc.vector.tensor_add(
                        ot[:], po[:], bob[:, eh * T_TILE:(eh + 1) * T_TILE])
                    nc.sync.dma_start(
                        out_sl[tc_i * P:(tc_i + 1) * P,
                               eh * T_TILE:(eh + 1) * T_TILE], ot[:])

    nc.compile()
    return nc


_NC = None


def _get_nc():
    global _NC
    if _NC is None:
        _NC = build_nc()
    return _NC


def _make_in_maps(inputs):
    x = np.asarray(inputs["x"], np.float32)
    adj = np.asarray(inputs["adj"], np.float32)
    Wq_f = np.asarray(inputs["Wq"], np.float32).astype(NP_BF16)
    bq_f = np.ascontiguousarray(np.asarray(inputs["bq"], np.float32))
    Wk_f = np.asarray(inputs["Wk"], np.float32).reshape(E, G, DH).sum(axis=1) \
        .astype(NP_BF16)
    bk_f = np.ascontiguousarray(
        np.asarray(inputs["bk"], np.float32).reshape(G, DH).sum(axis=0))
    Wv_f = np.asarray(inputs["Wv"], np.float32).reshape(E, G, DH).sum(axis=1) \
        .astype(NP_BF16)
    bv_f = np.ascontiguousarray(
        np.asarray(inputs["bv"], np.float32).reshape(G, DH).sum(axis=0))
    Wo_f = np.asarray(inputs["Wo"], np.float32).astype(NP_BF16)
    bo_f = np.ascontiguousarray(np.asarray(inputs["bo"], np.float32))

    in_maps = []
    for c in range(N_CORES):
        b = c // (N_CORES // B)
        tq = c % (N_CORES // B)
        in_maps.append({
            "x_sl": np.ascontiguousarray(
                x[b, tq * T_CORE:(tq + 1) * T_CORE, :].astype(NP_BF16)),
            "adj_b": np.ascontiguousarray(adj[b].astype(NP_BF16)),
            "Wq": Wq_f, "bq": bq_f, "Wk": Wk_f, "bk": bk_f,
            "Wv": Wv_f, "bv": bv_f, "Wo": Wo_f, "bo": bo_f,
        })
    return in_maps


def kernel(x, adj, Wq, bq, Wk, bk, Wv, bv, Wo, bo):
    inputs = {"x": x, "adj": adj, "Wq": Wq, "bq": bq, "Wk": Wk, "bk": bk,
              "Wv": Wv, "bv": bv, "Wo": Wo, "bo": bo}
    nc = _get_nc()
    in_maps = _make_in_maps(inputs)

    from concourse.bass_utils import run_bass_kernel_spmd
    res = run_bass_kernel_spmd(nc, in_maps, list(range(N_CORES)))

    out = np.empty((B, T, HID), np.float32)
    for c in range(N_CORES):
        b = c // (N_CORES // B)
        tq = c % (N_CORES // B)
        out[b, tq * T_CORE:(tq + 1) * T_CORE, :] = res.results[c]["out_sl"]
    return out


# revision 6
# speedup vs baseline: 1.1658x; 1.0519x over previous
"""Trainium2 Bass kernel for nn_CrossAttention (B=2, T=V=4096, 16 heads, d=64).

Math: the reference einsums contract the k/v group axis g, so
  weight = softmax((x@Wq) @ (adj @ sum_g Wk_g)^T / sqrt(64))
  out    = (weight @ (adj @ sum_g Wv_g)) @ Wo + bo
The group fold (sum over g of Wk/Wv columns) is done host-side on the
weights; all tensor-sized compute runs on device.

Sharding: 8 cores = (batch b, quarter of T). Each core takes t-rows
[tq*1024, (tq+1)*1024) of batch b, needs adj[b] (redundant across the 4
cores of the same b), and writes its own out slice. No collectives.

All matmul operands are bf16 (cast host-side for DRAM inputs); PSUM
accumulation stays fp32, softmax normalize + final output are fp32.

Structure (v2):
  B: 512-row adj stripes -> PE-transpose -> aT -> ONE combined K|V
     projection per stripe (Wk and Wv side by side in a [e,128]
     stationary): rows 0:64 -> K^T columns, rows 64:128 -> V^T stripe,
     which is PE-transposed again into vt ([v,65] + ones col).
  C0: x(t-half 0) -> xT -> q^T heads via Wq.
  D: per (t-half, head-pair): 32 v-blocks x (2 S matmuls into a
     [128,1024] PSUM tile, one Exp (scale=1/8, bf16 out), 2 P@V
     accumulations into O2[65,2,512]).  Row 64 of O2 = softmax sums.
     PSUM: S2 double-buffered (4 banks) + O2 (2) + proj (1) +
     transpose scratch (1) = 8 banks.
  Interleave: C1 (q^T for t-half 1) is emitted one micro-op per
     v-block during D(t-half 0); the first half of the out-projection
     during D(t-half 1).  E epilog does the rest.
"""

import numpy as np
import ml_dtypes

import concourse.bass as bass
import concourse.tile as tile
from concourse import bacc, mybir
from concourse.masks import make_identity

F32 = mybir.dt.float32
BF16 = mybir.dt.bfloat16
NP_BF16 = ml_dtypes.bfloat16

B = 2
T = 4096
V = 4096
E = 1024
HID = 1024
NH = 16
DH = 64
G = 4
N_CORES = 8
T_CORE = (B * T) // N_CORES  # 1024
P = 128

T_TILE = 512
ROW_G = 512           # rows per build stripe
SCALE = 1.0 / 8.0


def build_nc():
    EB = E // P                # 8
    DB = HID // P              # 8
    NVB = V // P               # 32
    NTT = T_CORE // T_TILE     # 2
    GC = ROW_G // P            # 4 chunks per stripe
    NSTRIPE_V = V // ROW_G     # 8

    nc = bacc.Bacc("TRN2", target_bir_lowering=False, debug=False,
                   num_devices=N_CORES)

    x_sl = nc.declare_dram_parameter("x_sl", [T_CORE, E], BF16, isOutput=False)
    adj_b = nc.declare_dram_parameter("adj_b", [V, E], BF16, isOutput=False)
    Wq = nc.declare_dram_parameter("Wq", [E, HID], BF16, isOutput=False)
    bq = nc.declare_dram_parameter("bq", [HID], F32, isOutput=False)
    Wkv = nc.declare_dram_parameter("Wkv", [E, P], BF16, isOutput=False)
    bk = nc.declare_dram_parameter("bk", [DH], F32, isOutput=False)
    bv = nc.declare_dram_parameter("bv", [DH], F32, isOutput=False)
    Wo = nc.declare_dram_parameter("Wo", [HID, HID], BF16, isOutput=False)
    bo = nc.declare_dram_parameter("bo", [HID], F32, isOutput=False)
    out_sl = nc.declare_dram_parameter("out_sl", [T_CORE, HID], F32,
                                       isOutput=True)
    sums_dram = nc.dram_tensor("sums_scratch", [NH, T_CORE], F32)

    def bcast_ap(param, n_part, n_free):
        a = param[:] if not isinstance(param, bass.AP) else param
        return bass.AP(tensor=a.tensor, offset=a.offset,
                       ap=[[0, n_part]] + list(a.ap))

    from contextlib import ExitStack
    with tile.TileContext(nc, pool_alloc_mode="queue") as tc, ExitStack() as st:
        consts = st.enter_context(tc.tile_pool(name="consts", bufs=1))
        persist = st.enter_context(tc.tile_pool(name="persist", bufs=1))
        # SBUF work pools.
        bw = st.enter_context(tc.tile_pool(name="bw", bufs=2))
        cw = st.enter_context(tc.tile_pool(name="cw", bufs=2))
        w1 = st.enter_context(tc.tile_pool(name="w1", bufs=1))
        dw = st.enter_context(tc.tile_pool(name="dw", bufs=3))
        dn = st.enter_context(tc.tile_pool(name="dn", bufs=2))
        ew = st.enter_context(tc.tile_pool(name="ew", bufs=2))

        ident = consts.tile([P, P], BF16)
        make_identity(nc, ident[:])
        bq_sb = consts.tile([P, DB], F32)
        nc.sync.dma_start(bq_sb[:], bq.rearrange("(db dp) -> dp db", dp=P))
        bk_sb = consts.tile([DH, 1], F32)
        nc.sync.dma_start(bk_sb[:], bk.rearrange("(a one) -> a one", one=1))
        bv_sb = consts.tile([P, 1], F32)
        nc.sync.dma_start(bv_sb[DH:P, :],
                          bv.rearrange("(a one) -> a one", one=1))
        bob = consts.tile([P, HID], F32)
        nc.gpsimd.dma_start(bob[:], bcast_ap(bo, P, HID))

        kT = persist.tile([P, V], BF16)
        vt = persist.tile([P, NVB, DH + 1], BF16)
        qT = persist.tile([P, NH, T_CORE], BF16)
        attnT = persist.tile([P, DB, T_CORE], BF16)
        nc.gpsimd.memset(kT[DH:P, :], 0.0)
        nc.gpsimd.memset(qT[DH:P, :, :], 0.0)
        nc.gpsimd.memset(vt[:, :, DH:DH + 1], 1.0)

        # Big weights loaded once, early (DMA overlaps the adj stream).
        Wq_sb = w1.tile([P, EB, HID], BF16)
        nc.sync.dma_start(Wq_sb[:], Wq.rearrange("(eb ep) d -> ep eb d", ep=P))
        Wo_sb = w1.tile([P, DB, HID], BF16)
        nc.sync.dma_start(Wo_sb[:], Wo.rearrange("(kb kp) e -> kp kb e", kp=P))
        Wkv_sb = w1.tile([P, EB, P], BF16)
        nc.sync.dma_start(Wkv_sb[:],
                          Wkv.rearrange("(eb ep) d -> ep eb d", ep=P))

        def transpose_block(dst_ap, src_ap, tr_pool):
            """PE-transpose a [128,128] bf16 block src -> dst (SBUF)."""
            ptr = tr_pool.tile([P, P], BF16, tag="ptr", name="ptr")
            nc.tensor.transpose(ptr[:], src_ap, ident[:])
            nc.vector.tensor_copy(dst_ap, ptr[:])

        # ---- Phase B: K^T and V~ from adj (8 stripes of 512 rows) ----
        def emit_b_stripe(sv, mm_pool, tr_pool):
            r0 = sv * ROW_G
            adj_in = bw.tile([P, GC, E], BF16, tag="row_in")
            nc.sync.dma_start(
                adj_in[:],
                adj_b[r0:r0 + ROW_G, :].rearrange("(c p) e -> p c e", p=P))
            aT = bw.tile([P, EB, ROW_G], BF16, tag="aT")
            for eb in range(EB):
                for cc in range(GC):
                    transpose_block(aT[:, eb, cc * P:(cc + 1) * P],
                                    adj_in[:, cc, eb * P:(eb + 1) * P],
                                    tr_pool)
            pkv = mm_pool.tile([P, ROW_G], F32, tag="proj", name="pkv")
            for eb in range(EB):
                nc.tensor.matmul(pkv[:], Wkv_sb[:, eb, :], aT[:, eb, :],
                                 start=(eb == 0), stop=(eb == EB - 1))
            nc.vector.tensor_scalar_add(kT[0:DH, r0:r0 + ROW_G],
                                        pkv[0:DH, :], bk_sb[:])
            vtmp = bw.tile([P, ROW_G], BF16, tag="vtmp")
            nc.vector.tensor_scalar_add(vtmp[DH:P, :], pkv[DH:P, :],
                                        bv_sb[DH:P, :])
            for cc in range(GC):
                vb = (r0 + cc * P) // P
                pvt = tr_pool.tile([P, DH], BF16, tag="pvt", name="pvt")
                nc.tensor.transpose(pvt[:], vtmp[DH:P, cc * P:(cc + 1) * P],
                                    ident[DH:P, DH:P])
                nc.vector.tensor_copy(vt[:, vb, 0:DH], pvt[:])

        # ---- Phase C: q^T for one t-half (emitted inline for tt=0,
        # as background tasks for tt=1) ----
        def c_tasks(tt, mm_pool, tr_pool):
            ts0 = tt * T_TILE
            xT = cw.tile([P, EB, T_TILE], BF16, tag="xT", bufs=1)
            # DMA the whole 512-row half immediately (it overlaps whatever
            # else is running; the dependent transposes are deferred).
            x_in = cw.tile([P, GC, E], BF16, tag="x_in", bufs=1)
            nc.sync.dma_start(
                x_in[:],
                x_sl[ts0:ts0 + T_TILE, :].rearrange("(c p) e -> p c e", p=P))
            tasks = []
            for eb in range(EB):
                for cc in range(GC):
                    def tr_task(eb=eb, cc=cc):
                        transpose_block(xT[:, eb, cc * P:(cc + 1) * P],
                                        x_in[:, cc, eb * P:(eb + 1) * P],
                                        tr_pool)
                    tasks.append(tr_task)
            for db in range(DB):
                pq_box = {}
                def mm_task(db=db, eb=0, pq_box=pq_box):
                    pq_box["pq"] = mm_pool.tile([P, T_TILE], F32, tag="proj", name="pq")
                    nc.tensor.matmul(pq_box["pq"][:],
                                     Wq_sb[:, 0, db * P:(db + 1) * P],
                                     xT[:, 0, :], start=True, stop=False)
                tasks.append(mm_task)
                for eb in range(1, EB):
                    def mm_task2(db=db, eb=eb, pq_box=pq_box):
                        nc.tensor.matmul(pq_box["pq"][:],
                                         Wq_sb[:, eb, db * P:(db + 1) * P],
                                         xT[:, eb, :],
                                         start=False, stop=(eb == EB - 1))
                    tasks.append(mm_task2)
                def evac_task(db=db, pq_box=pq_box, ts0=ts0):
                    pq = pq_box["pq"]
                    nc.vector.tensor_scalar_add(
                        qT[0:DH, 2 * db, ts0:ts0 + T_TILE],
                        pq[0:DH, :], bq_sb[0:DH, db:db + 1])
                    qtmp = cw.tile([P, T_TILE], BF16, tag="qtmp")
                    nc.vector.tensor_scalar_add(
                        qtmp[DH:P, :], pq[DH:P, :], bq_sb[DH:P, db:db + 1])
                    nc.gpsimd.dma_start(
                        qT[0:DH, 2 * db + 1, ts0:ts0 + T_TILE],
                        qtmp[DH:P, :])
                tasks.append(evac_task)
            return tasks

        # ---- Phase E: out-projection tasks (16 groups of 9) ----
        def e_tasks(tc_lo, tc_hi, mm_pool):
            tasks = []
            for tc_i in range(tc_lo, tc_hi):
                for eh in range(HID // T_TILE):
                    po_box = {}
                    def mm0(tc_i=tc_i, eh=eh, po_box=po_box):
                        po_box["po"] = mm_pool.tile([P, T_TILE], F32, tag="proj", name="po")
                        nc.tensor.matmul(
                            po_box["po"][:],
                            attnT[:, 0, tc_i * P:(tc_i + 1) * P],
                            Wo_sb[:, 0, eh * T_TILE:(eh + 1) * T_TILE],
                            start=True, stop=False)
                    tasks.append(mm0)
                    for kb in range(1, DB):
                        def mmk(tc_i=tc_i, eh=eh, kb=kb, po_box=po_box):
                            nc.tensor.matmul(
                                po_box["po"][:],
                                attnT[:, kb, tc_i * P:(tc_i + 1) * P],
                                Wo_sb[:, kb, eh * T_TILE:(eh + 1) * T_TILE],
                                start=False, stop=(kb == DB - 1))
                        tasks.append(mmk)
                    def evac(tc_i=tc_i, eh=eh, po_box=po_box):
                        ot = ew.tile([P, T_TILE], F32, tag="ot")
                        nc.vector.tensor_add(
                            ot[:], po_box["po"][:],
                            bob[:, eh * T_TILE:(eh + 1) * T_TILE])
                        nc.sync.dma_start(
                            out_sl[tc_i * P:(tc_i + 1) * P,
                                   eh * T_TILE:(eh + 1) * T_TILE], ot[:])
                    tasks.append(evac)
            return tasks

        # ---- Emit prolog: B stripes, then C0 inline ----
        with (
            tc.tile_pool(name="mmpB", bufs=2, space="PSUM") as mmpB,
            tc.tile_pool(name="trpB", bufs=3, space="PSUM") as trpB,
        ):
            for sv in range(NSTRIPE_V):
                emit_b_stripe(sv, mmpB, trpB)
            for t in c_tasks(0, mmpB, trpB):
                t()

        # ---- Phase D with background-task interleave ----
        bg = []

        def drain(n=1):
            for _ in range(n):
                if bg:
                    bg.pop(0)()

        dctx = ExitStack()
        sp2 = dctx.enter_context(tc.tile_pool(name="sp2", bufs=2, space="PSUM"))
        op2 = dctx.enter_context(tc.tile_pool(name="op2", bufs=1, space="PSUM"))
        mmp1 = dctx.enter_context(tc.tile_pool(name="mmp1", bufs=1, space="PSUM"))
        trp1 = dctx.enter_context(tc.tile_pool(name="trp1", bufs=1, space="PSUM"))

        for tt in range(NTT):
            ts0 = tt * T_TILE
            if tt == 0:
                bg.extend(c_tasks(1, mmp1, trp1))
            else:
                bg.extend(e_tasks(0, T_TILE // P, mmp1))
            for g in range(NH // 2):
                h0, h1 = 2 * g, 2 * g + 1
                O2 = op2.tile([DH + 1, 2, T_TILE], F32, tag="O2")
                for vb in range(NVB):
                    S2 = sp2.tile([P, 2 * T_TILE], F32, tag="S2")
                    nc.tensor.matmul(S2[:, 0:T_TILE],
                                     kT[:, vb * P:(vb + 1) * P],
                                     qT[:, h0, ts0:ts0 + T_TILE],
                                     start=True, stop=True)
                    nc.tensor.matmul(S2[:, T_TILE:2 * T_TILE],
                                     kT[:, vb * P:(vb + 1) * P],
                                     qT[:, h1, ts0:ts0 + T_TILE],
                                     start=True, stop=True)
                    P2 = dw.tile([P, 2 * T_TILE], BF16, tag="P2")
                    nc.scalar.activation(P2[:], S2[:],
                                         mybir.ActivationFunctionType.Exp,
                                         scale=SCALE)
                    for hi in range(2):
                        nc.tensor.matmul(
                            O2[:, hi, :], vt[:, vb, :],
                            P2[:, hi * T_TILE:(hi + 1) * T_TILE],
                            start=(vb == 0), stop=(vb == NVB - 1),
                            skip_group_check=True)
                    drain(1)
                # normalize this head pair
                onorm = dn.tile([DH + 1, 2, T_TILE], F32, tag="onorm")
                nc.vector.tensor_copy(onorm[:], O2[:])
                nc.gpsimd.dma_start(
                    sums_dram[h0:h0 + 2, ts0:ts0 + T_TILE],
                    onorm[DH:DH + 1, :, :])
                sbc = dn.tile([DH, 2, T_TILE], F32, tag="sbc")
                nc.gpsimd.dma_start(
                    sbc[:],
                    bcast_ap(sums_dram[h0:h0 + 2, ts0:ts0 + T_TILE],
                             DH, 2 * T_TILE))
                rec = dn.tile([DH, 2, T_TILE], F32, tag="rec")
                nc.vector.reciprocal_approx_fast(rec[:], sbc[:])
                nc.vector.tensor_mul(attnT[0:DH, g, ts0:ts0 + T_TILE],
                                     onorm[0:DH, 0, :], rec[:, 0, :])
                nrm = dn.tile([DH, T_TILE], BF16, tag="nrm")
                nc.vector.tensor_mul(nrm[:], onorm[0:DH, 1, :], rec[:, 1, :])
                nc.gpsimd.dma_start(attnT[DH:P, g, ts0:ts0 + T_TILE], nrm[:])
        # drain any leftover background tasks
        while bg:
            bg.pop(0)()
        dctx.close()

        # ---- E epilog: second t-half of the out-projection ----
        with tc.tile_pool(name="mmpE", bufs=2, space="PSUM") as mmpE:
            for t in e_tasks(T_TILE // P, T_CORE // P, mmpE):
                t()

    nc.compile()
    return nc


_NC = None


def _get_nc():
    global _NC
    if _NC is None:
        _NC = build_nc()
    return _NC


def _make_in_maps(inputs):
    x = np.asarray(inputs["x"], np.float32)
    adj = np.asarray(inputs["adj"], np.float32)
    Wq_f = np.asarray(inputs["Wq"], np.float32).astype(NP_BF16)
    bq_f = np.ascontiguousarray(np.asarray(inputs["bq"], np.float32))
    Wk_f = np.asarray(inputs["Wk"], np.float32).reshape(E, G, DH).sum(axis=1)
    bk_f = np.ascontiguousarray(
        np.asarray(inputs["bk"], np.float32).reshape(G, DH).sum(axis=0))
    Wv_f = np.asarray(inputs["Wv"], np.float32).reshape(E, G, DH).sum(axis=1)
    bv_f = np.ascontiguousarray(
        np.asarray(inputs["bv"], np.float32).reshape(G, DH).sum(axis=0))
    Wkv_f = np.ascontiguousarray(
        np.concatenate([Wk_f, Wv_f], axis=1).astype(NP_BF16))
    Wo_f = np.asarray(inputs["Wo"], np.float32).astype(NP_BF16)
    bo_f = np.ascontiguousarray(np.asarray(inputs["bo"], np.float32))

    in_maps = []
    for c in range(N_CORES):
        b = c // (N_CORES // B)
        tq = c % (N_CORES // B)
        in_maps.append({
            "x_sl": np.ascontiguousarray(
                x[b, tq * T_CORE:(tq + 1) * T_CORE, :].astype(NP_BF16)),
            "adj_b": np.ascontiguousarray(adj[b].astype(NP_BF16)),
            "Wq": Wq_f, "bq": bq_f, "Wkv": Wkv_f, "bk": bk_f,
            "bv": bv_f, "Wo": Wo_f, "bo": bo_f,
        })
    return in_maps


def kernel(x, adj, Wq, bq, Wk, bk, Wv, bv, Wo, bo):
    inputs = {"x": x, "adj": adj, "Wq": Wq, "bq": bq, "Wk": Wk, "bk": bk,
              "Wv": Wv, "bv": bv, "Wo": Wo, "bo": bo}
    nc = _get_nc()
    in_maps = _make_in_maps(inputs)

    from concourse.bass_utils import run_bass_kernel_spmd
    res = run_bass_kernel_spmd(nc, in_maps, list(range(N_CORES)))

    out = np.empty((B, T, HID), np.float32)
    for c in range(N_CORES):
        b = c // (N_CORES // B)
        tq = c % (N_CORES // B)
        out[b, tq * T_CORE:(tq + 1) * T_CORE, :] = res.results[c]["out_sl"]
    return out


# revision 10
# speedup vs baseline: 1.2549x; 1.0765x over previous
"""Trainium2 Bass kernel for nn_CrossAttention (B=2, T=V=4096, 16 heads, d=64).

Math: the reference einsums contract the k/v group axis g, so
  weight = softmax((x@Wq) @ (adj @ sum_g Wk_g)^T / sqrt(64))
  out    = (weight @ (adj @ sum_g Wv_g)) @ Wo + bo
The group fold (sum over g of Wk/Wv columns) is done host-side on the
weights; all tensor-sized compute runs on device.

Sharding: 8 cores = (batch b, quarter of T). Each core takes t-rows
[tq*1024, (tq+1)*1024) of batch b, needs adj[b] (redundant across the 4
cores of the same b), and writes its own out slice. No collectives.

All matmul operands are bf16 (cast host-side for DRAM inputs); PSUM
accumulation stays fp32, softmax normalize + final output are fp32.

Structure (v2):
  B: 512-row adj stripes -> PE-transpose -> aT -> ONE combined K|V
     projection per stripe (Wk and Wv side by side in a [e,128]
     stationary): rows 0:64 -> K^T columns, rows 64:128 -> V^T stripe,
     which is PE-transposed again into vt ([v,65] + ones col).
  C0: x(t-half 0) -> xT -> q^T heads via Wq.
  D: per (t-half, head-pair): 32 v-blocks x (2 S matmuls into a
     [128,1024] PSUM tile, one Exp (scale=1/8, bf16 out), 2 P@V
     accumulations into O2[65,2,512]).  Row 64 of O2 = softmax sums.
     PSUM: S2 double-buffered (4 banks) + O2 (2) + proj (1) +
     transpose scratch (1) = 8 banks.
  Interleave: C1 (q^T for t-half 1) is emitted one micro-op per
     v-block during D(t-half 0); the first half of the out-projection
     during D(t-half 1).  E epilog does the rest.
"""

import numpy as np
import ml_dtypes

import concourse.bass as bass
import concourse.tile as tile
from concourse import bacc, mybir
from concourse.masks import make_identity

F32 = mybir.dt.float32
BF16 = mybir.dt.bfloat16
NP_BF16 = ml_dtypes.bfloat16

B = 2
T = 4096
V = 4096
E = 1024
HID = 1024
NH = 16
DH = 64
G = 4
N_CORES = 8
T_CORE = (B * T) // N_CORES  # 1024
P = 128

T_TILE = 512
ROW_G = 512           # rows per build stripe
SCALE = 1.0 / 8.0


def build_nc():
    EB = E // P                # 8
    DB = HID // P              # 8
    NVB = V // P               # 32
    NTT = T_CORE // T_TILE     # 2
    GC = ROW_G // P            # 4 chunks per stripe
    NSTRIPE_V = V // ROW_G     # 8

    nc = bacc.Bacc("TRN2", target_bir_lowering=False, debug=False,
                   num_devices=N_CORES)

    x_sl = nc.declare_dram_parameter("x_sl", [T_CORE, E], BF16, isOutput=False)
    adj_b = nc.declare_dram_parameter("adj_b", [V, E], BF16, isOutput=False)
    Wq = nc.declare_dram_parameter("Wq", [E, HID], BF16, isOutput=False)
    bq = nc.declare_dram_parameter("bq", [HID], F32, isOutput=False)
    Wkv = nc.declare_dram_parameter("Wkv", [E, P], BF16, isOutput=False)
    bk = nc.declare_dram_parameter("bk", [DH], F32, isOutput=False)
    bv = nc.declare_dram_parameter("bv", [DH], F32, isOutput=False)
    Wo = nc.declare_dram_parameter("Wo", [HID, HID], BF16, isOutput=False)
    bo = nc.declare_dram_parameter("bo", [HID], F32, isOutput=False)
    out_sl = nc.declare_dram_parameter("out_sl", [T_CORE, HID], F32,
                                       isOutput=True)
    sums_dram = nc.dram_tensor("sums_scratch", [NH, T_CORE], F32)

    def bcast_ap(param, n_part, n_free):
        a = param[:] if not isinstance(param, bass.AP) else param
        return bass.AP(tensor=a.tensor, offset=a.offset,
                       ap=[[0, n_part]] + list(a.ap))

    from contextlib import ExitStack
    with tile.TileContext(nc, pool_alloc_mode="queue") as tc, ExitStack() as st:
        consts = st.enter_context(tc.tile_pool(name="consts", bufs=1))
        persist = st.enter_context(tc.tile_pool(name="persist", bufs=1))
        # SBUF work pools.
        bw = st.enter_context(tc.tile_pool(name="bw", bufs=2))
        cw = st.enter_context(tc.tile_pool(name="cw", bufs=2))
        w1 = st.enter_context(tc.tile_pool(name="w1", bufs=1))
        dw = st.enter_context(tc.tile_pool(name="dw", bufs=3))
        dn = st.enter_context(tc.tile_pool(name="dn", bufs=2))
        ew = st.enter_context(tc.tile_pool(name="ew", bufs=2))

        ident = consts.tile([P, P], BF16)
        make_identity(nc, ident[:])
        bq_sb = consts.tile([P, DB], F32)
        nc.sync.dma_start(bq_sb[:], bq.rearrange("(db dp) -> dp db", dp=P))
        bk_sb = consts.tile([DH, 1], F32)
        nc.sync.dma_start(bk_sb[:], bk.rearrange("(a one) -> a one", one=1))
        bv_sb = consts.tile([P, 1], F32)
        nc.sync.dma_start(bv_sb[DH:P, :],
                          bv.rearrange("(a one) -> a one", one=1))
        bob = consts.tile([P, HID], F32)
        nc.gpsimd.dma_start(bob[:], bcast_ap(bo, P, HID))

        kT = persist.tile([P, V], BF16)
        vt = persist.tile([P, NVB, DH + 1], BF16)
        qT = persist.tile([P, NH, T_CORE], BF16)
        attnT = persist.tile([P, DB, T_CORE], BF16)
        nc.gpsimd.memset(kT[DH:P, :], 0.0)
        nc.gpsimd.memset(qT[DH:P, :, :], 0.0)
        nc.gpsimd.memset(vt[:, :, DH:DH + 1], 1.0)

        # Weight tiles; DMAs issued interleaved with the adj stream below
        # so the first stripes aren't queued behind 4MB of weights.
        Wq_sb = w1.tile([P, EB, HID], BF16)
        Wo_sb = w1.tile([P, DB, HID], BF16)
        Wkv_sb = w1.tile([P, EB, P], BF16)

        def transpose_block(dst_ap, src_ap, tr_pool):
            """PE-transpose a [128,128] bf16 block src -> dst (SBUF)."""
            ptr = tr_pool.tile([P, P], BF16, tag="ptr", name="ptr")
            nc.tensor.transpose(ptr[:], src_ap, ident[:])
            nc.vector.tensor_copy(dst_ap, ptr[:])

        # ---- Phase B: K^T and V~ from adj (8 stripes of 512 rows) ----
        def b_stripe_dma(sv):
            r0 = sv * ROW_G
            adj_in = bw.tile([P, GC, E], BF16, tag="row_in", name="adj_in")
            nc.sync.dma_start(
                adj_in[:],
                adj_b[r0:r0 + ROW_G, :].rearrange("(c p) e -> p c e", p=P))
            return adj_in

        def emit_b_stripe(sv, adj_in, mm_pool, tr_pool):
            r0 = sv * ROW_G
            aT = bw.tile([P, EB, ROW_G], BF16, tag="aT")
            for eb in range(EB):
                for cc in range(GC):
                    transpose_block(aT[:, eb, cc * P:(cc + 1) * P],
                                    adj_in[:, cc, eb * P:(eb + 1) * P],
                                    tr_pool)
            pkv = mm_pool.tile([P, ROW_G], F32, tag="proj", name="pkv")
            for eb in range(EB):
                nc.tensor.matmul(pkv[:], Wkv_sb[:, eb, :], aT[:, eb, :],
                                 start=(eb == 0), stop=(eb == EB - 1))
            nc.vector.tensor_scalar_add(kT[0:DH, r0:r0 + ROW_G],
                                        pkv[0:DH, :], bk_sb[:])
            vtmp = bw.tile([P, ROW_G], BF16, tag="vtmp")
            nc.vector.tensor_scalar_add(vtmp[DH:P, :], pkv[DH:P, :],
                                        bv_sb[DH:P, :])
            for cc in range(GC):
                vb = (r0 + cc * P) // P
                pvt = tr_pool.tile([P, DH], BF16, tag="pvt", name="pvt")
                nc.tensor.transpose(pvt[:], vtmp[DH:P, cc * P:(cc + 1) * P],
                                    ident[DH:P, DH:P])
                nc.vector.tensor_copy(vt[:, vb, 0:DH], pvt[:])

        # ---- Phase C: q^T for one t-half (emitted inline for tt=0,
        # as background tasks for tt=1) ----
        def c_tasks(tt, mm_pool, tr_pool):
            ts0 = tt * T_TILE
            xT = cw.tile([P, EB, T_TILE], BF16, tag="xT", bufs=1)
            # DMA the whole 512-row half immediately (it overlaps whatever
            # else is running; the dependent transposes are deferred).
            x_in = cw.tile([P, GC, E], BF16, tag="x_in", bufs=1)
            nc.sync.dma_start(
                x_in[:],
                x_sl[ts0:ts0 + T_TILE, :].rearrange("(c p) e -> p c e", p=P))
            tasks = []
            for eb in range(EB):
                for cc in range(GC):
                    def tr_task(eb=eb, cc=cc):
                        transpose_block(xT[:, eb, cc * P:(cc + 1) * P],
                                        x_in[:, cc, eb * P:(eb + 1) * P],
                                        tr_pool)
                    tasks.append(tr_task)
            for db in range(DB):
                pq_box = {}
                def mm_task(db=db, eb=0, pq_box=pq_box):
                    pq_box["pq"] = mm_pool.tile([P, T_TILE], F32, tag="proj", name="pq")
                    nc.tensor.matmul(pq_box["pq"][:],
                                     Wq_sb[:, 0, db * P:(db + 1) * P],
                                     xT[:, 0, :], start=True, stop=False)
                tasks.append(mm_task)
                for eb in range(1, EB):
                    def mm_task2(db=db, eb=eb, pq_box=pq_box):
                        nc.tensor.matmul(pq_box["pq"][:],
                                         Wq_sb[:, eb, db * P:(db + 1) * P],
                                         xT[:, eb, :],
                                         start=False, stop=(eb == EB - 1))
                    tasks.append(mm_task2)
                def evac_task(db=db, pq_box=pq_box, ts0=ts0):
                    pq = pq_box["pq"]
                    nc.vector.tensor_scalar_add(
                        qT[0:DH, 2 * db, ts0:ts0 + T_TILE],
                        pq[0:DH, :], bq_sb[0:DH, db:db + 1])
                    qtmp = cw.tile([P, T_TILE], BF16, tag="qtmp", bufs=1)
                    nc.vector.tensor_scalar_add(
                        qtmp[DH:P, :], pq[DH:P, :], bq_sb[DH:P, db:db + 1])
                    nc.gpsimd.dma_start(
                        qT[0:DH, 2 * db + 1, ts0:ts0 + T_TILE],
                        qtmp[DH:P, :])
                tasks.append(evac_task)
            return tasks

        # ---- Phase E: out-projection tasks (16 groups of 9) ----
        def e_tasks(tc_lo, tc_hi, mm_pool):
            tasks = []
            for tc_i in range(tc_lo, tc_hi):
                for eh in range(HID // T_TILE):
                    po_box = {}
                    def mm0(tc_i=tc_i, eh=eh, po_box=po_box):
                        po_box["po"] = mm_pool.tile([P, T_TILE], F32, tag="proj", name="po")
                        nc.tensor.matmul(
                            po_box["po"][:],
                            attnT[:, 0, tc_i * P:(tc_i + 1) * P],
                            Wo_sb[:, 0, eh * T_TILE:(eh + 1) * T_TILE],
                            start=True, stop=False)
                    tasks.append(mm0)
                    for kb in range(1, DB):
                        def mmk(tc_i=tc_i, eh=eh, kb=kb, po_box=po_box):
                            nc.tensor.matmul(
                                po_box["po"][:],
                                attnT[:, kb, tc_i * P:(tc_i + 1) * P],
                                Wo_sb[:, kb, eh * T_TILE:(eh + 1) * T_TILE],
                                start=False, stop=(kb == DB - 1))
                        tasks.append(mmk)
                    def evac(tc_i=tc_i, eh=eh, po_box=po_box):
                        ot = ew.tile([P, T_TILE], F32, tag="ot")
                        nc.vector.tensor_add(
                            ot[:], po_box["po"][:],
                            bob[:, eh * T_TILE:(eh + 1) * T_TILE])
                        nc.sync.dma_start(
                            out_sl[tc_i * P:(tc_i + 1) * P,
                                   eh * T_TILE:(eh + 1) * T_TILE], ot[:])
                    tasks.append(evac)
            return tasks

        # ---- Emit prolog: B stripes, then C0 inline ----
        with (
            tc.tile_pool(name="mmpB", bufs=2, space="PSUM") as mmpB,
            tc.tile_pool(name="trpB", bufs=3, space="PSUM") as trpB,
        ):
            adj0 = b_stripe_dma(0)
            adj1 = b_stripe_dma(1)
            nc.sync.dma_start(Wkv_sb[:],
                              Wkv.rearrange("(eb ep) d -> ep eb d", ep=P))
            c0 = c_tasks(0, mmpB, trpB)  # issues the x(tt0) DMA now
            emit_b_stripe(0, adj0, mmpB, trpB)
            nc.sync.dma_start(Wq_sb[:],
                              Wq.rearrange("(eb ep) d -> ep eb d", ep=P))
            emit_b_stripe(1, adj1, mmpB, trpB)
            for sv in range(2, NSTRIPE_V):
                adj_in = b_stripe_dma(sv)
                emit_b_stripe(sv, adj_in, mmpB, trpB)
            nc.sync.dma_start(Wo_sb[:],
                              Wo.rearrange("(kb kp) e -> kp kb e", kp=P))
            for t in c0:
                t()

        # ---- Phase D with background-task interleave ----
        bg = []

        def drain(n=1):
            for _ in range(n):
                if bg:
                    bg.pop(0)()

        pend = []
        pend_pvs = [0]
        dctx = ExitStack()
        sp2 = dctx.enter_context(tc.tile_pool(name="sp2", bufs=2, space="PSUM"))
        op2 = dctx.enter_context(tc.tile_pool(name="op2", bufs=1, space="PSUM"))
        mmp1 = dctx.enter_context(tc.tile_pool(name="mmp1", bufs=1, space="PSUM"))
        trp1 = dctx.enter_context(tc.tile_pool(name="trp1", bufs=1, space="PSUM"))

        for tt in range(NTT):
            ts0 = tt * T_TILE
            if tt == 0:
                bg.extend(c_tasks(1, mmp1, trp1))
            else:
                bg.extend(e_tasks(0, T_TILE // P, mmp1))
            for g in range(NH // 2):
                h0, h1 = 2 * g, 2 * g + 1
                O2 = op2.tile([DH + 1, 2, T_TILE], F32, tag="O2", name="O2")
                for vb in range(NVB):
                    S2 = sp2.tile([P, 2 * T_TILE], F32, tag="S2")
                    nc.tensor.matmul(S2[:, 0:T_TILE],
                                     kT[:, vb * P:(vb + 1) * P],
                                     qT[:, h0, ts0:ts0 + T_TILE],
                                     start=True, stop=True)
                    nc.tensor.matmul(S2[:, T_TILE:2 * T_TILE],
                                     kT[:, vb * P:(vb + 1) * P],
                                     qT[:, h1, ts0:ts0 + T_TILE],
                                     start=True, stop=True)
                    P2 = dw.tile([P, 2 * T_TILE], BF16, tag="P2", bufs=5,
                                 name="P2")
                    nc.scalar.activation(P2[:], S2[:],
                                         mybir.ActivationFunctionType.Exp,
                                         scale=SCALE)
                    for hi in range(2):
                        def pv_op(O2=O2, vb2=vb, hi=hi, P2=P2):
                            nc.tensor.matmul(
                                O2[:, hi, :], vt[:, vb2, :],
                                P2[:, hi * T_TILE:(hi + 1) * T_TILE],
                                start=(vb2 == 0), stop=(vb2 == NVB - 1),
                                skip_group_check=True)
                        pend.append(("pv", pv_op))
                        pend_pvs[0] += 1
                    if vb == NVB - 1:
                        def norm_op(O2=O2, g=g, h0=h0, ts0=ts0):
                            onorm = dn.tile([DH + 1, 2, T_TILE], F32,
                                            tag="onorm", bufs=1, name="onorm")
                            nc.vector.tensor_copy(onorm[:], O2[:])
                            nc.gpsimd.dma_start(
                                sums_dram[h0:h0 + 2, ts0:ts0 + T_TILE],
                                onorm[DH:DH + 1, :, :])
                            sbc = dn.tile([DH, 2, T_TILE], F32,
                                          tag="sbc", bufs=1, name="sbc")
                            nc.gpsimd.dma_start(
                                sbc[:],
                                bcast_ap(sums_dram[h0:h0 + 2,
                                                   ts0:ts0 + T_TILE],
                                         DH, 2 * T_TILE))
                            nc.vector.reciprocal_approx_fast(sbc[:], sbc[:])
                            nc.vector.tensor_mul(
                                attnT[0:DH, g, ts0:ts0 + T_TILE],
                                onorm[0:DH, 0, :], sbc[:, 0, :])
                            nrm = dn.tile([DH, T_TILE], BF16,
                                          tag="nrm", bufs=1, name="nrm")
                            nc.vector.tensor_mul(nrm[:], onorm[0:DH, 1, :],
                                                 sbc[:, 1, :])
                            nc.gpsimd.dma_start(
                                attnT[DH:P, g, ts0:ts0 + T_TILE], nrm[:])
                        pend.append(("norm", norm_op))
                    # Emit up to 2 pending PV matmuls per iteration, keeping
                    # >=4 queued so a PV trails its exp by ~2 v-blocks and
                    # the in-order PE queue never stalls on ACT.
                    npv = 0
                    while pend and npv < 2:
                        kind, op = pend[0]
                        if kind == "pv":
                            if pend_pvs[0] <= 4:
                                break
                            pend_pvs[0] -= 1
                            npv += 1
                        pend.pop(0)
                        op()
                    drain(1)
        # flush pending PV / normalize work, then leftover bg tasks
        while pend:
            pend.pop(0)[1]()
        while bg:
            bg.pop(0)()
        dctx.close()

        # ---- E epilog: second t-half of the out-projection ----
        with tc.tile_pool(name="mmpE", bufs=3, space="PSUM") as mmpE:
            for t in e_tasks(T_TILE // P, T_CORE // P, mmpE):
                t()

    nc.compile()
    return nc


_NC = None


def _get_nc():
    global _NC
    if _NC is None:
        _NC = build_nc()
    return _NC


def _make_in_maps(inputs):
    x = np.asarray(inputs["x"], np.float32)
    adj = np.asarray(inputs["adj"], np.float32)
    Wq_f = np.asarray(inputs["Wq"], np.float32).astype(NP_BF16)
    bq_f = np.ascontiguousarray(np.asarray(inputs["bq"], np.float32))
    Wk_f = np.asarray(inputs["Wk"], np.float32).reshape(E, G, DH).sum(axis=1)
    bk_f = np.ascontiguousarray(
        np.asarray(inputs["bk"], np.float32).reshape(G, DH).sum(axis=0))
    Wv_f = np.asarray(inputs["Wv"], np.float32).reshape(E, G, DH).sum(axis=1)
    bv_f = np.ascontiguousarray(
        np.asarray(inputs["bv"], np.float32).reshape(G, DH).sum(axis=0))
    Wkv_f = np.ascontiguousarray(
        np.concatenate([Wk_f, Wv_f], axis=1).astype(NP_BF16))
    Wo_f = np.asarray(inputs["Wo"], np.float32).astype(NP_BF16)
    bo_f = np.ascontiguousarray(np.asarray(inputs["bo"], np.float32))

    in_maps = []
    for c in range(N_CORES):
        b = c // (N_CORES // B)
        tq = c % (N_CORES // B)
        in_maps.append({
            "x_sl": np.ascontiguousarray(
                x[b, tq * T_CORE:(tq + 1) * T_CORE, :].astype(NP_BF16)),
            "adj_b": np.ascontiguousarray(adj[b].astype(NP_BF16)),
            "Wq": Wq_f, "bq": bq_f, "Wkv": Wkv_f, "bk": bk_f,
            "bv": bv_f, "Wo": Wo_f, "bo": bo_f,
        })
    return in_maps


def kernel(x, adj, Wq, bq, Wk, bk, Wv, bv, Wo, bo):
    inputs = {"x": x, "adj": adj, "Wq": Wq, "bq": bq, "Wk": Wk, "bk": bk,
              "Wv": Wv, "bv": bv, "Wo": Wo, "bo": bo}
    nc = _get_nc()
    in_maps = _make_in_maps(inputs)

    from concourse.bass_utils import run_bass_kernel_spmd
    res = run_bass_kernel_spmd(nc, in_maps, list(range(N_CORES)))

    out = np.empty((B, T, HID), np.float32)
    for c in range(N_CORES):
        b = c // (N_CORES // B)
        tq = c % (N_CORES // B)
        out[b, tq * T_CORE:(tq + 1) * T_CORE, :] = res.results[c]["out_sl"]
    return out


# revision 11
# speedup vs baseline: 1.2751x; 1.0161x over previous
"""Trainium2 Bass kernel for nn_CrossAttention (B=2, T=V=4096, 16 heads, d=64).

Math: the reference einsums contract the k/v group axis g, so
  weight = softmax((x@Wq) @ (adj @ sum_g Wk_g)^T / sqrt(64))
  out    = (weight @ (adj @ sum_g Wv_g)) @ Wo + bo
The group fold (sum over g of Wk/Wv columns) is done host-side on the
weights; all tensor-sized compute runs on device.

Sharding: 8 cores = (batch b, quarter of T). Each core takes t-rows
[tq*1024, (tq+1)*1024) of batch b, needs adj[b] (redundant across the 4
cores of the same b), and writes its own out slice. No collectives.

All matmul operands are bf16 (cast host-side for DRAM inputs); PSUM
accumulation stays fp32, softmax normalize + final output are fp32.

Structure (v2):
  B: 512-row adj stripes -> PE-transpose -> aT -> ONE combined K|V
     projection per stripe (Wk and Wv side by side in a [e,128]
     stationary): rows 0:64 -> K^T columns, rows 64:128 -> V^T stripe,
     which is PE-transposed again into vt ([v,65] + ones col).
  C0: x(t-half 0) -> xT -> q^T heads via Wq.
  D: per (t-half, head-pair): 32 v-blocks x (2 S matmuls into a
     [128,1024] PSUM tile, one Exp (scale=1/8, bf16 out), 2 P@V
     accumulations into O2[65,2,512]).  Row 64 of O2 = softmax sums.
     PSUM: S2 double-buffered (4 banks) + O2 (2) + proj (1) +
     transpose scratch (1) = 8 banks.
  Interleave: C1 (q^T for t-half 1) is emitted one micro-op per
     v-block during D(t-half 0); the first half of the out-projection
     during D(t-half 1).  E epilog does the rest.
"""

import numpy as np
import ml_dtypes

import concourse.bass as bass
import concourse.tile as tile
from concourse import bacc, mybir
from concourse.masks import make_identity

F32 = mybir.dt.float32
BF16 = mybir.dt.bfloat16
NP_BF16 = ml_dtypes.bfloat16

B = 2
T = 4096
V = 4096
E = 1024
HID = 1024
NH = 16
DH = 64
G = 4
N_CORES = 8
T_CORE = (B * T) // N_CORES  # 1024
P = 128

T_TILE = 512
ROW_G = 512           # rows per build stripe
SCALE = 1.0 / 8.0


def build_nc():
    EB = E // P                # 8
    DB = HID // P              # 8
    NVB = V // P               # 32
    NTT = T_CORE // T_TILE     # 2
    GC = ROW_G // P            # 4 chunks per stripe
    NSTRIPE_V = V // ROW_G     # 8

    nc = bacc.Bacc("TRN2", target_bir_lowering=False, debug=False,
                   num_devices=N_CORES)

    x_sl = nc.declare_dram_parameter("x_sl", [T_CORE, E], BF16, isOutput=False)
    adj_b = nc.declare_dram_parameter("adj_b", [V, E], BF16, isOutput=False)
    Wq = nc.declare_dram_parameter("Wq", [E, HID], BF16, isOutput=False)
    bq = nc.declare_dram_parameter("bq", [HID], F32, isOutput=False)
    Wkv = nc.declare_dram_parameter("Wkv", [E, P], BF16, isOutput=False)
    bk = nc.declare_dram_parameter("bk", [DH], F32, isOutput=False)
    bv = nc.declare_dram_parameter("bv", [DH], F32, isOutput=False)
    Wo = nc.declare_dram_parameter("Wo", [HID, HID], BF16, isOutput=False)
    bo = nc.declare_dram_parameter("bo", [HID], F32, isOutput=False)
    out_sl = nc.declare_dram_parameter("out_sl", [T_CORE, HID], F32,
                                       isOutput=True)
    sums_dram = nc.dram_tensor("sums_scratch", [NH, T_CORE], F32)

    def bcast_ap(param, n_part, n_free):
        a = param[:] if not isinstance(param, bass.AP) else param
        return bass.AP(tensor=a.tensor, offset=a.offset,
                       ap=[[0, n_part]] + list(a.ap))

    from contextlib import ExitStack
    with tile.TileContext(nc, pool_alloc_mode="queue") as tc, ExitStack() as st:
        consts = st.enter_context(tc.tile_pool(name="consts", bufs=1))
        persist = st.enter_context(tc.tile_pool(name="persist", bufs=1))
        # SBUF work pools.
        bw = st.enter_context(tc.tile_pool(name="bw", bufs=2))
        cw = st.enter_context(tc.tile_pool(name="cw", bufs=2))
        w1 = st.enter_context(tc.tile_pool(name="w1", bufs=1))
        dw = st.enter_context(tc.tile_pool(name="dw", bufs=3))
        dn = st.enter_context(tc.tile_pool(name="dn", bufs=2))
        ew = st.enter_context(tc.tile_pool(name="ew", bufs=2))

        ident = consts.tile([P, P], BF16)
        make_identity(nc, ident[:])
        bq_sb = consts.tile([P, DB], F32)
        nc.sync.dma_start(bq_sb[:], bq.rearrange("(db dp) -> dp db", dp=P))
        bk_sb = consts.tile([DH, 1], F32)
        nc.sync.dma_start(bk_sb[:], bk.rearrange("(a one) -> a one", one=1))
        bv_sb = consts.tile([P, 1], F32)
        nc.sync.dma_start(bv_sb[DH:P, :],
                          bv.rearrange("(a one) -> a one", one=1))
        bob = consts.tile([P, HID], F32)
        nc.gpsimd.dma_start(bob[:], bcast_ap(bo, P, HID))

        kT = persist.tile([P, V], BF16)
        vt = persist.tile([P, NVB, DH + 1], BF16)
        qT = persist.tile([P, NH, T_CORE], BF16)
        attnT = persist.tile([P, DB, T_CORE], BF16)
        nc.gpsimd.memset(kT[DH:P, :], 0.0)
        nc.gpsimd.memset(qT[DH:P, :, :], 0.0)
        nc.gpsimd.memset(vt[:, :, DH:DH + 1], 1.0)

        # Weight tiles; DMAs issued interleaved with the adj stream below
        # so the first stripes aren't queued behind 4MB of weights.
        Wq_sb = w1.tile([P, EB, HID], BF16)
        Wo_sb = w1.tile([P, DB, HID], BF16)
        Wkv_sb = w1.tile([P, EB, P], BF16)

        def transpose_block(dst_ap, src_ap, tr_pool, use_act=False):
            """PE-transpose a [128,128] bf16 block src -> dst (SBUF)."""
            ptr = tr_pool.tile([P, P], BF16, tag="ptr", name="ptr")
            nc.tensor.transpose(ptr[:], src_ap, ident[:])
            if use_act:
                nc.scalar.copy(dst_ap, ptr[:])
            else:
                nc.vector.tensor_copy(dst_ap, ptr[:])

        # ---- Phase B: K^T and V~ from adj (8 stripes of 512 rows) ----
        def b_stripe_dma(sv):
            r0 = sv * ROW_G
            adj_in = bw.tile([P, GC, E], BF16, tag="row_in", name="adj_in")
            nc.sync.dma_start(
                adj_in[:],
                adj_b[r0:r0 + ROW_G, :].rearrange("(c p) e -> p c e", p=P))
            return adj_in

        def emit_b_stripe(sv, adj_in, mm_pool, tr_pool):
            r0 = sv * ROW_G
            aT = bw.tile([P, EB, ROW_G], BF16, tag="aT")
            for eb in range(EB):
                for cc in range(GC):
                    transpose_block(aT[:, eb, cc * P:(cc + 1) * P],
                                    adj_in[:, cc, eb * P:(eb + 1) * P],
                                    tr_pool, use_act=(eb + cc) % 2 == 0)
            pkv = mm_pool.tile([P, ROW_G], F32, tag="proj", name="pkv")
            for eb in range(EB):
                nc.tensor.matmul(pkv[:], Wkv_sb[:, eb, :], aT[:, eb, :],
                                 start=(eb == 0), stop=(eb == EB - 1))
            nc.scalar.activation(kT[0:DH, r0:r0 + ROW_G], pkv[0:DH, :],
                                 mybir.ActivationFunctionType.Identity,
                                 bias=bk_sb[:])
            vtmp = bw.tile([P, ROW_G], BF16, tag="vtmp")
            nc.vector.tensor_scalar_add(vtmp[DH:P, :], pkv[DH:P, :],
                                        bv_sb[DH:P, :])
            for cc in range(GC):
                vb = (r0 + cc * P) // P
                pvt = tr_pool.tile([P, DH], BF16, tag="pvt", name="pvt")
                nc.tensor.transpose(pvt[:], vtmp[DH:P, cc * P:(cc + 1) * P],
                                    ident[DH:P, DH:P])
                nc.vector.tensor_copy(vt[:, vb, 0:DH], pvt[:])

        # ---- Phase C: q^T for one t-half (emitted inline for tt=0,
        # as background tasks for tt=1) ----
        def c_tasks(tt, mm_pool, tr_pool, use_act=False):
            ts0 = tt * T_TILE
            xT = cw.tile([P, EB, T_TILE], BF16, tag="xT", bufs=1)
            # DMA the whole 512-row half immediately (it overlaps whatever
            # else is running; the dependent transposes are deferred).
            x_in = cw.tile([P, GC, E], BF16, tag="x_in", bufs=1)
            nc.sync.dma_start(
                x_in[:],
                x_sl[ts0:ts0 + T_TILE, :].rearrange("(c p) e -> p c e", p=P))
            tasks = []
            for eb in range(EB):
                for cc in range(GC):
                    def tr_task(eb=eb, cc=cc):
                        transpose_block(xT[:, eb, cc * P:(cc + 1) * P],
                                        x_in[:, cc, eb * P:(eb + 1) * P],
                                        tr_pool,
                                        use_act=(use_act and
                                                 (eb + cc) % 2 == 0))
                    tasks.append(tr_task)
            for db in range(DB):
                pq_box = {}
                def mm_task(db=db, eb=0, pq_box=pq_box):
                    pq_box["pq"] = mm_pool.tile([P, T_TILE], F32, tag="proj", name="pq")
                    nc.tensor.matmul(pq_box["pq"][:],
                                     Wq_sb[:, 0, db * P:(db + 1) * P],
                                     xT[:, 0, :], start=True, stop=False)
                tasks.append(mm_task)
                for eb in range(1, EB):
                    def mm_task2(db=db, eb=eb, pq_box=pq_box):
                        nc.tensor.matmul(pq_box["pq"][:],
                                         Wq_sb[:, eb, db * P:(db + 1) * P],
                                         xT[:, eb, :],
                                         start=False, stop=(eb == EB - 1))
                    tasks.append(mm_task2)
                def evac_task(db=db, pq_box=pq_box, ts0=ts0):
                    pq = pq_box["pq"]
                    if use_act:
                        nc.scalar.activation(
                            qT[0:DH, 2 * db, ts0:ts0 + T_TILE],
                            pq[0:DH, :],
                            mybir.ActivationFunctionType.Identity,
                            bias=bq_sb[0:DH, db:db + 1])
                    else:
                        nc.vector.tensor_scalar_add(
                            qT[0:DH, 2 * db, ts0:ts0 + T_TILE],
                            pq[0:DH, :], bq_sb[0:DH, db:db + 1])
                    qtmp = cw.tile([P, T_TILE], BF16, tag="qtmp", bufs=1)
                    nc.vector.tensor_scalar_add(
                        qtmp[DH:P, :], pq[DH:P, :], bq_sb[DH:P, db:db + 1])
                    nc.gpsimd.dma_start(
                        qT[0:DH, 2 * db + 1, ts0:ts0 + T_TILE],
                        qtmp[DH:P, :])
                tasks.append(evac_task)
            return tasks

        # ---- Phase E: out-projection tasks (16 groups of 9) ----
        def e_tasks(tc_lo, tc_hi, mm_pool):
            tasks = []
            for tc_i in range(tc_lo, tc_hi):
                for eh in range(HID // T_TILE):
                    po_box = {}
                    def mm0(tc_i=tc_i, eh=eh, po_box=po_box):
                        po_box["po"] = mm_pool.tile([P, T_TILE], F32, tag="proj", name="po")
                        nc.tensor.matmul(
                            po_box["po"][:],
                            attnT[:, 0, tc_i * P:(tc_i + 1) * P],
                            Wo_sb[:, 0, eh * T_TILE:(eh + 1) * T_TILE],
                            start=True, stop=False)
                    tasks.append(mm0)
                    for kb in range(1, DB):
                        def mmk(tc_i=tc_i, eh=eh, kb=kb, po_box=po_box):
                            nc.tensor.matmul(
                                po_box["po"][:],
                                attnT[:, kb, tc_i * P:(tc_i + 1) * P],
                                Wo_sb[:, kb, eh * T_TILE:(eh + 1) * T_TILE],
                                start=False, stop=(kb == DB - 1))
                        tasks.append(mmk)
                    def evac(tc_i=tc_i, eh=eh, po_box=po_box):
                        ot = ew.tile([P, T_TILE], F32, tag="ot")
                        nc.vector.tensor_add(
                            ot[:], po_box["po"][:],
                            bob[:, eh * T_TILE:(eh + 1) * T_TILE])
                        nc.sync.dma_start(
                            out_sl[tc_i * P:(tc_i + 1) * P,
                                   eh * T_TILE:(eh + 1) * T_TILE], ot[:])
                    tasks.append(evac)
            return tasks

        # ---- Emit prolog: B stripes, then C0 inline ----
        with (
            tc.tile_pool(name="mmpB", bufs=2, space="PSUM") as mmpB,
            tc.tile_pool(name="trpB", bufs=3, space="PSUM") as trpB,
        ):
            adj0 = b_stripe_dma(0)
            adj1 = b_stripe_dma(1)
            nc.sync.dma_start(Wkv_sb[:],
                              Wkv.rearrange("(eb ep) d -> ep eb d", ep=P))
            c0 = c_tasks(0, mmpB, trpB, use_act=True)  # issues the x(tt0) DMA now
            emit_b_stripe(0, adj0, mmpB, trpB)
            nc.sync.dma_start(Wq_sb[:],
                              Wq.rearrange("(eb ep) d -> ep eb d", ep=P))
            emit_b_stripe(1, adj1, mmpB, trpB)
            for sv in range(2, NSTRIPE_V):
                adj_in = b_stripe_dma(sv)
                emit_b_stripe(sv, adj_in, mmpB, trpB)
            nc.sync.dma_start(Wo_sb[:],
                              Wo.rearrange("(kb kp) e -> kp kb e", kp=P))
            for t in c0:
                t()

        # ---- Phase D with background-task interleave ----
        bg = []

        def drain(n=1):
            for _ in range(n):
                if bg:
                    bg.pop(0)()

        pend = []
        pend_pvs = [0]
        dctx = ExitStack()
        sp2 = dctx.enter_context(tc.tile_pool(name="sp2", bufs=2, space="PSUM"))
        op2 = dctx.enter_context(tc.tile_pool(name="op2", bufs=1, space="PSUM"))
        mmp1 = dctx.enter_context(tc.tile_pool(name="mmp1", bufs=1, space="PSUM"))
        trp1 = dctx.enter_context(tc.tile_pool(name="trp1", bufs=1, space="PSUM"))

        for tt in range(NTT):
            ts0 = tt * T_TILE
            if tt == 0:
                bg.extend(c_tasks(1, mmp1, trp1))
            else:
                bg.extend(e_tasks(0, T_TILE // P, mmp1))
            for g in range(NH // 2):
                h0, h1 = 2 * g, 2 * g + 1
                O2 = op2.tile([DH + 1, 2, T_TILE], F32, tag="O2", name="O2")
                for vb in range(NVB):
                    S2 = sp2.tile([P, 2 * T_TILE], F32, tag="S2")
                    nc.tensor.matmul(S2[:, 0:T_TILE],
                                     kT[:, vb * P:(vb + 1) * P],
                                     qT[:, h0, ts0:ts0 + T_TILE],
                                     start=True, stop=True)
                    nc.tensor.matmul(S2[:, T_TILE:2 * T_TILE],
                                     kT[:, vb * P:(vb + 1) * P],
                                     qT[:, h1, ts0:ts0 + T_TILE],
                                     start=True, stop=True)
                    P2 = dw.tile([P, 2 * T_TILE], BF16, tag="P2", bufs=5,
                                 name="P2")
                    nc.scalar.activation(P2[:], S2[:],
                                         mybir.ActivationFunctionType.Exp,
                                         scale=SCALE)
                    for hi in range(2):
                        def pv_op(O2=O2, vb2=vb, hi=hi, P2=P2):
                            nc.tensor.matmul(
                                O2[:, hi, :], vt[:, vb2, :],
                                P2[:, hi * T_TILE:(hi + 1) * T_TILE],
                                start=(vb2 == 0), stop=(vb2 == NVB - 1),
                                skip_group_check=True)
                        pend.append(("pv", pv_op))
                        pend_pvs[0] += 1
                    if vb == NVB - 1:
                        def norm_op(O2=O2, g=g, h0=h0, ts0=ts0):
                            onorm = dn.tile([DH + 1, 2, T_TILE], F32,
                                            tag="onorm", bufs=1, name="onorm")
                            nc.vector.tensor_copy(onorm[:], O2[:])
                            nc.gpsimd.dma_start(
                                sums_dram[h0:h0 + 2, ts0:ts0 + T_TILE],
                                onorm[DH:DH + 1, :, :])
                            sbc = dn.tile([DH, 2, T_TILE], F32,
                                          tag="sbc", bufs=1, name="sbc")
                            nc.gpsimd.dma_start(
                                sbc[:],
                                bcast_ap(sums_dram[h0:h0 + 2,
                                                   ts0:ts0 + T_TILE],
                                         DH, 2 * T_TILE))
                            nc.vector.reciprocal_approx_fast(sbc[:], sbc[:])
                            nc.vector.tensor_mul(
                                attnT[0:DH, g, ts0:ts0 + T_TILE],
                                onorm[0:DH, 0, :], sbc[:, 0, :])
                            nrm = dn.tile([DH, T_TILE], BF16,
                                          tag="nrm", bufs=1, name="nrm")
                            nc.vector.tensor_mul(nrm[:], onorm[0:DH, 1, :],
                                                 sbc[:, 1, :])
                            nc.gpsimd.dma_start(
                                attnT[DH:P, g, ts0:ts0 + T_TILE], nrm[:])
                        pend.append(("norm", norm_op))
                    # Emit up to 2 pending PV matmuls per iteration, keeping
                    # >=4 queued so a PV trails its exp by ~2 v-blocks and
                    # the in-order PE queue never stalls on ACT.
                    npv = 0
                    while pend and npv < 2:
                        kind, op = pend[0]
                        if kind == "pv":
                            if pend_pvs[0] <= 4:
                                break
                            pend_pvs[0] -= 1
                            npv += 1
                        pend.pop(0)
                        op()
                    drain(1)
        # flush pending PV / normalize work, then leftover bg tasks
        while pend:
            pend.pop(0)[1]()
        while bg:
            bg.pop(0)()
        dctx.close()

        # ---- E epilog: second t-half of the out-projection ----
        with tc.tile_pool(name="mmpE", bufs=3, space="PSUM") as mmpE:
            for t in e_tasks(T_TILE // P, T_CORE // P, mmpE):
                t()

    nc.compile()
    return nc


_NC = None


def _get_nc():
    global _NC
    if _NC is None:
        _NC = build_nc()
    return _NC


def _make_in_maps(inputs):
    x = np.asarray(inputs["x"], np.float32)
    adj = np.asarray(inputs["adj"], np.float32)
    Wq_f = np.asarray(inputs["Wq"], np.float32).astype(NP_BF16)
    bq_f = np.ascontiguousarray(np.asarray(inputs["bq"], np.float32))
    Wk_f = np.asarray(inputs["Wk"], np.float32).reshape(E, G, DH).sum(axis=1)
    bk_f = np.ascontiguousarray(
        np.asarray(inputs["bk"], np.float32).reshape(G, DH).sum(axis=0))
    Wv_f = np.asarray(inputs["Wv"], np.float32).reshape(E, G, DH).sum(axis=1)
    bv_f = np.ascontiguousarray(
        np.asarray(inputs["bv"], np.float32).reshape(G, DH).sum(axis=0))
    Wkv_f = np.ascontiguousarray(
        np.concatenate([Wk_f, Wv_f], axis=1).astype(NP_BF16))
    Wo_f = np.asarray(inputs["Wo"], np.float32).astype(NP_BF16)
    bo_f = np.ascontiguousarray(np.asarray(inputs["bo"], np.float32))

    in_maps = []
    for c in range(N_CORES):
        b = c // (N_CORES // B)
        tq = c % (N_CORES // B)
        in_maps.append({
            "x_sl": np.ascontiguousarray(
                x[b, tq * T_CORE:(tq + 1) * T_CORE, :].astype(NP_BF16)),
            "adj_b": np.ascontiguousarray(adj[b].astype(NP_BF16)),
            "Wq": Wq_f, "bq": bq_f, "Wkv": Wkv_f, "bk": bk_f,
            "bv": bv_f, "Wo": Wo_f, "bo": bo_f,
        })
    return in_maps


def kernel(x, adj, Wq, bq, Wk, bk, Wv, bv, Wo, bo):
    inputs = {"x": x, "adj": adj, "Wq": Wq, "bq": bq, "Wk": Wk, "bk": bk,
              "Wv": Wv, "bv": bv, "Wo": Wo, "bo": bo}
    nc = _get_nc()
    in_maps = _make_in_maps(inputs)

    from concourse.bass_utils import run_bass_kernel_spmd
    res = run_bass_kernel_spmd(nc, in_maps, list(range(N_CORES)))

    out = np.empty((B, T, HID), np.float32)
    for c in range(N_CORES):
        b = c // (N_CORES // B)
        tq = c % (N_CORES // B)
        out[b, tq * T_CORE:(tq + 1) * T_CORE, :] = res.results[c]["out_sl"]
    return out
